# revision 36
# baseline (speedup 1.0000x reference)
"""Multi-head attention (B=512,S=64,D=1024,H=16) on 8 trn2 NeuronCores.

Strategy: pure data-parallel over the batch dim — each core gets 64 batches
(4096 tokens) and runs the full fused MHA layer locally; no collectives.

Per-core dataflow (token chunks of 512 = 8 batches):
  xT [1024, tok] bf16 arrives pre-transposed (feature-major) from the host
  qT = Wq.T @ xT (feature-major); K lands as per-batch block-diagonal tiles
  kd (diag for even batches, antidiag for odd); V lands as per-batch vband
  tiles [h_up | ones | h_dn | ones] via a normal-order matmul for even heads
  and a token-swapped matmul for odd heads (same total PE rows).

  scoresT for BOTH heads of a pair in ONE matmul per (batch, pair):
    lhsT = kd block [128,128] (block-diagonal, full 128-contraction),
    rhs = qT slice [128, 64] -> es [128 = h_up ktok | h_dn ktok, 64 qtok]
  ctx for both heads in ONE matmul per (tile-half, pair):
    lhsT = es [128, 64], rhs = vband block [128, 130] (block-diag with ones
    cols) -> [64 qtok, 130] = [ctx_up | Z_up | ctx_dn | Z_dn]; two halves
    pack one PSUM bank via tile_position col 0/64 (same full row-strip).
  normalize with per-partition reciprocal pairs; ctxT via batched PE
  transpose; out = gelu(ctx @ Wo) token-major -> DRAM.

This halves the attention matmul count (and score rows) vs the quadrant
scheme: the PE's HAM power budget (~536us of full-speed execution, then
4/8 duty-cycling) makes every saved PE cycle worth ~1.5-2x in wall time.

The emission order software-pipelines chunks: window ch emits chunk ch's
dense QKV projections interleaved with chunk ch-1's attention and chunk
ch-3's output projection (the delay keeps the tail full of dense matmuls).

PSUM packing rule (hardware): two concurrent matmuls may share a PSUM bank
only if they use the same array row-strip (same operand base partition) or
a strict diagonal (row,col) placement; different row-strips draining into
one bank is fatal. All sharing here uses full-128 row strips.
PSUM budget (bank-granular): proj 2 + sc 2 + cx 2 + tp 2 = 8 banks.
"""

import sys

sys.path.insert(0, "/opt/trn_rl_repo")

import numpy as np
import ml_dtypes

import concourse.bass as bass
import concourse.tile as tile
from concourse import mybir
from concourse.bass_utils import run_bass_kernel_spmd
from concourse.masks import make_identity

F32 = mybir.dt.float32
BF = mybir.dt.bfloat16

B, S, D, H = 512, 64, 1024, 16
DH = D // H  # 64
NCORES = 8
BL = B // NCORES  # 64 batches per core
NTOK = BL * S  # 4096 tokens per core
CHUNK = 512  # tokens per pipeline chunk (8 batches)
NCH = NTOK // CHUNK  # 8
TT = CHUNK // 128  # 4 token-tiles per chunk
KT = D // 128  # 8 d-tiles
SCALE = 1.0 / np.sqrt(np.float32(D))  # 1/32


def _split_multiwait(nc, limit=1):
    """walrus can emit at most one sync-wait per instruction; TileContext's
    tail drain carries one wait per touched processor. Hoist extras onto
    chained NOPs."""
    f = nc.m.functions[0]
    for blk in f.blocks:
        new_insts = []
        for inst in blk.instructions:
            si = inst.sync_info
            if si is not None and len(si.on_wait) > limit:
                extra = si.on_wait[:-limit]
                keep = si.on_wait[-limit:]
                for i, w in enumerate(extra):
                    nop = mybir.InstNoOp(
                        name=f"{inst.name}-waitsplit{i}",
                        sync_info=mybir.SyncInfo(on_wait=[w], on_update=[]),
                        bass_nofuse=True,
                        ins=[],
                        outs=[],
                    )
                    nop.engine = inst.engine
                    new_insts.append(nop)
                si.on_wait[:] = keep
            new_insts.append(inst)
        blk.instructions[:] = new_insts


def _interleave(a, b):
    """Merge two unit lists round-robin, proportionally to their lengths."""
    out = []
    ia = ib = 0
    la, lb = len(a), len(b)
    while ia < la or ib < lb:
        if ib >= lb or (ia < la and ia * lb <= ib * la):
            out.append(a[ia])
            ia += 1
        else:
            out.append(b[ib])
            ib += 1
    return out


def build(split_waits=True):
    nc = bass.Bass("TRN2", debug=False, num_devices=NCORES)

    # x arrives pre-transposed (feature-major) from the host: [D, NTOK]
    x_d = nc.declare_dram_parameter("x", [D, NTOK], BF, isOutput=False)
    w_d = {}
    b_d = {}
    for nm in ("wq", "wk", "wv", "wo"):
        w_d[nm] = nc.declare_dram_parameter(f"{nm}_w", [D, D], BF, isOutput=False)
        b_d[nm] = nc.declare_dram_parameter(f"{nm}_b", [D], F32, isOutput=False)
    out_d = nc.declare_dram_parameter("out", [NTOK, D], F32, isOutput=True)

    with tile.TileContext(nc) as tc:
        with (
            tc.tile_pool(name="weights", bufs=1) as wpool,
            tc.tile_pool(name="consts", bufs=1) as cpool,
            tc.tile_pool(name="feat", bufs=2) as fpool,
            tc.tile_pool(name="attn", bufs=4) as apool,
            tc.tile_pool(name="outb", bufs=2) as opool,
            tc.tile_pool(name="psum", bufs=2, space="PSUM") as ppool,
        ):
            wt = {nm: [None] * KT for nm in ("wq", "wk", "wv", "wo")}
            biases = {}
            consts = {}
            wtiles = {}

            def unit_load_weight(nm, h=0, halves=1):
                """dma_start for 1/halves of the [D,D] matrix: k-tile k lands
                at cols k*D of a merged [128, KT*D] tile (contiguous 2KB
                runs). Split loads let the first k-tiles' matmuls start while
                the rest of the matrix is still in flight."""

                def f():
                    if nm not in wtiles:
                        wb = wpool.tile(
                            [128, KT * D], BF, tag=f"w_{nm}", name=f"w{nm}"
                        )
                        wtiles[nm] = wb
                        for k in range(KT):
                            wt[nm][k] = wb[:, k * D : (k + 1) * D]
                    wb = wtiles[nm]
                    hk = KT // halves
                    nc.sync.dma_start(
                        out=wb[:, h * hk * D : (h + 1) * hk * D].rearrange(
                            "p (k c) -> p k c", c=D
                        ),
                        in_=w_d[nm][h * hk * 128 : (h + 1) * hk * 128, :].rearrange(
                            "(k p) c -> p k c", p=128
                        ),
                    )

                return f

            def unit_biases():
                def f():
                    # per-partition (feature-major) bias layout for q/k evac
                    for nm in ("wq", "wk"):
                        bt = cpool.tile([128, KT], F32, tag=f"{nm}_pb", name=f"{nm}_pb")
                        nc.sync.dma_start(
                            out=bt[:], in_=b_d[nm][:].rearrange("(m p) -> p m", p=128)
                        )
                        biases[nm] = bt
                    # broadcast-to-all-partitions bias tiles for v/o via a
                    # partition-stride-0 DMA read (same row replicated 128x)
                    for nm in ("wv", "wo"):
                        bc = cpool.tile([128, D], F32, tag=f"{nm}_bc", name=f"{nm}_bc")
                        nc.sync.dma_start(
                            out=bc[:],
                            in_=b_d[nm][:].unsqueeze(0).broadcast_to((128, D)),
                        )
                        biases[nm] = bc

                return f

            live = {}  # per-chunk tiles handed from stage A to stage B

            def batched_transpose(src_slices, dst_ap):
                """Transpose up to 4 [128,128] src slices into one [128,512]
                PSUM tile (disjoint col blocks, same row-strip => legal bank
                sharing), then drain with a single DVE copy whose dst AP
                (shape [128, n, 128]) scatters the blocks to strided columns
                of the merged destination tile."""
                n = len(src_slices)
                ps = ppool.tile([128, 128 * n], BF, tag="tp", bufs=2, name="ps_tp")
                for j, src in enumerate(src_slices):
                    nc.tensor.transpose(
                        ps[:, j * 128 : (j + 1) * 128], src, consts["identity"]
                    )
                src_ap = ps[:].rearrange("p (b c) -> p b c", c=128)
                nc.vector.tensor_copy(out=dst_ap, in_=src_ap)

            def stage_a_units(ch):
                """X load, then QKV projections for chunk ch."""
                tok0 = ch * CHUNK
                st = live.setdefault(ch, {})

                def u_x(h=0, halves=1):
                    """dma_start for 1/halves of the chunk's pre-transposed x:
                    k-tile k lands at cols k*CHUNK of the merged
                    [128, KT*CHUNK] tile (contiguous 1KB runs)."""

                    def f():
                        if "xT" not in st:
                            st["xT"] = fpool.tile(
                                [128, KT * CHUNK], BF, tag="xT", name="xT"
                            )
                        hk = KT // halves
                        nc.sync.dma_start(
                            out=st["xT"][:, h * hk * CHUNK : (h + 1) * hk * CHUNK]
                            .rearrange("p (k t) -> p k t", t=CHUNK),
                            in_=x_d[
                                h * hk * 128 : (h + 1) * hk * 128,
                                tok0 : tok0 + CHUNK,
                            ].rearrange("(k p) t -> p k t", p=128),
                        )

                    return f

                def xT(k):
                    return st["xT"][:, k * CHUNK : (k + 1) * CHUNK]

                def u_q(m):
                    def f():
                        if "qT" not in st:
                            st["qT"] = [
                                fpool.tile([128, CHUNK], BF, tag=f"qT{i}", name=f"qT{i}")
                                for i in range(KT)
                            ]
                        ps = ppool.tile([128, CHUNK], F32, tag="proj", bufs=2, name="ps_q")
                        for k in range(KT):
                            nc.tensor.matmul(
                                ps[:],
                                lhsT=wt["wq"][k][:, m * 128 : (m + 1) * 128],
                                rhs=xT(k),
                                start=(k == 0),
                                stop=(k == KT - 1),
                            )
                        nc.scalar.activation(
                            out=st["qT"][m][:],
                            in_=ps[:],
                            func=mybir.ActivationFunctionType.Identity,
                            bias=biases["wq"][:, m : m + 1],
                        )

                    return f

                def u_k(m):
                    """K projection for head-pair m, evacuated into the
                    per-batch block-diagonal kd layout: even batches diag
                    (h_even up / h_odd down), odd batches antidiag. The four
                    strided evac ops run on the (otherwise idle) Pool engine.
                    Off-diagonal zeros come from a one-time buffer memset."""

                    def f():
                        if "kd" not in st:
                            st["kd"] = [
                                fpool.tile(
                                    [128, 8 * 128], BF, tag=f"kd{i}", name=f"kd{i}"
                                )
                                for i in range(KT)
                            ]
                            if ch < 2:
                                for i in range(KT):
                                    nc.gpsimd.memset(st["kd"][i][:], 0.0)
                        ps = ppool.tile([128, CHUNK], F32, tag="proj", bufs=2, name="ps_k")
                        for k in range(KT):
                            nc.tensor.matmul(
                                ps[:],
                                lhsT=wt["wk"][k][:, m * 128 : (m + 1) * 128],
                                rhs=xT(k),
                                start=(k == 0),
                                stop=(k == KT - 1),
                            )
                        kd3 = st["kd"][m].rearrange("p (b c) -> p b c", c=128)
                        ps3 = ps[:].rearrange("p (b t) -> p b t", t=64)
                        bk = biases["wk"][:, m : m + 1]
                        A = mybir.AluOpType.add
                        ID = mybir.ActivationFunctionType.Identity
                        # GPSIMD cannot read PSUM: split the four strided
                        # evacs between ACT (with fused bias) and DVE.
                        nc.scalar.activation(
                            out=kd3[0:64, 0::2, 0:64], in_=ps3[0:64, 0::2, :],
                            func=ID, bias=bk[0:64],
                        )
                        nc.scalar.activation(
                            out=kd3[64:128, 0::2, 64:128], in_=ps3[64:128, 0::2, :],
                            func=ID, bias=bk[64:128],
                        )
                        nc.vector.tensor_scalar(
                            out=kd3[64:128, 1::2, 0:64], in0=ps3[64:128, 1::2, :],
                            scalar1=bk[64:128], scalar2=None, op0=A,
                        )
                        nc.vector.tensor_scalar(
                            out=kd3[0:64, 1::2, 64:128], in0=ps3[0:64, 1::2, :],
                            scalar1=bk[0:64], scalar2=None, op0=A,
                        )

                    return f

                def _ensure_vband():
                    if "vb" not in st:
                        st["vb"] = [
                            apool.tile(
                                [128, 8 * 130], BF, tag=f"vb{b}", name=f"vb{b}", bufs=2
                            )
                            for b in range(8)
                        ]
                        if ch < 2:
                            for b in range(8):
                                vb3 = st["vb"][b].rearrange("p (t c) -> p t c", c=130)
                                nc.gpsimd.memset(st["vb"][b][:], 0.0)
                                nc.gpsimd.memset(vb3[0:64, :, 64:65], 1.0)
                                nc.gpsimd.memset(vb3[64:128, :, 129:130], 1.0)

                def u_v(t, n):
                    """V projection for token-tile t, feature half n. Even
                    heads evac lane-locked straight into vband (batch parity
                    matches its partition half); odd heads evac lane-locked
                    into a staging tile and are then partition-swapped into
                    the opposite vband halves by an SBUF->SBUF DMA (DMA is
                    not lane-locked)."""

                    def f():
                        _ensure_vband()
                        if "vstg" not in st:
                            st["vstg"] = [
                                apool.tile(
                                    [128, 8 * DH], BF, tag="vs", name=f"vs{i}",
                                    bufs=2,
                                )
                                for i in range(TT)
                            ]
                        ps = ppool.tile([128, CHUNK], F32, tag="proj", bufs=2, name="ps_v")
                        for k in range(KT):
                            nc.tensor.matmul(
                                ps[:],
                                lhsT=xT(k)[:, t * 128 : (t + 1) * 128],
                                rhs=wt["wv"][k][:, n * 512 : (n + 1) * 512],
                                start=(k == 0),
                                stop=(k == KT - 1),
                            )
                        bias3 = biases["wv"][:, n * 512 : (n + 1) * 512].rearrange(
                            "p (h c) -> p h c", c=DH
                        )
                        ps3 = ps[:].rearrange("p (h c) -> p h c", c=DH)
                        tp = slice(n * 4, n * 4 + 4)  # head-pair block range
                        vbE = st["vb"][2 * t].rearrange("p (t c) -> p t c", c=130)
                        vbO = st["vb"][2 * t + 1].rearrange("p (t c) -> p t c", c=130)
                        stg = st["vstg"][t].rearrange("p (t c) -> p t c", c=DH)
                        A = mybir.AluOpType.add
                        # even heads (0::2 within this half): direct
                        nc.vector.tensor_tensor(
                            out=vbE[0:64, tp, 0:64], in0=ps3[0:64, 0::2, :],
                            in1=bias3[0:64, 0::2, :], op=A,
                        )
                        nc.vector.tensor_tensor(
                            out=vbO[64:128, tp, 65:129], in0=ps3[64:128, 0::2, :],
                            in1=bias3[64:128, 0::2, :], op=A,
                        )
                        # odd heads: to staging (same partitions)
                        nc.vector.tensor_tensor(
                            out=stg[0:64, tp, :], in0=ps3[0:64, 1::2, :],
                            in1=bias3[0:64, 1::2, :], op=A,
                        )
                        nc.vector.tensor_tensor(
                            out=stg[64:128, tp, :], in0=ps3[64:128, 1::2, :],
                            in1=bias3[64:128, 1::2, :], op=A,
                        )
                        if n == 1:
                            # both halves staged: partition-swap via DMA
                            stg_full = st["vstg"][t].rearrange(
                                "p (t c) -> p t c", c=DH
                            )
                            nc.sync.dma_start(
                                out=vbE[64:128, :, 65:129], in_=stg_full[0:64, :, :]
                            )
                            nc.sync.dma_start(
                                out=vbO[0:64, :, 0:64], in_=stg_full[64:128, :, :]
                            )

                    return f

                proj = []
                for m in range(KT):
                    proj.append(u_q(m))
                    proj.append(u_k(m))
                for t in range(TT):
                    proj.append(u_v(t, False))
                    proj.append(u_v(t, True))
                return {
                    "x": [u_x()],
                    "x_split": [u_x(0, 2), u_x(1, 2)],
                    "q": [u_q(m) for m in range(KT)],
                    "k": [u_k(m) for m in range(KT)],
                    "v": [u_v(t, odd) for t in range(TT) for odd in (False, True)],
                    "proj": proj,
                }

            def attn_core_units(ch, with_out=False):
                """Attention for chunk ch, software-pipelined per token-tile:
                the score pair (t+1) is emitted before ctx(t) so the ACT exp
                latency hides behind the next pair's score matmuls."""
                st = live[ch]
                es_tiles = {}

                def u_scores(b, t):
                    def f():
                        es = apool.tile([128, 64], BF, tag="expS", name="es", bufs=6)
                        es_tiles[(b, t)] = es
                        ps_s = ppool.tile([128, 64], F32, tag="sc", bufs=2, name="ps_s")
                        nc.tensor.matmul(
                            ps_s[:],
                            lhsT=st["kd"][t][:, b * 128 : (b + 1) * 128],
                            rhs=st["qT"][t][:, b * 64 : (b + 1) * 64],
                            start=True,
                            stop=True,
                        )
                        nc.scalar.activation(
                            out=es[:],
                            in_=ps_s[:],
                            func=mybir.ActivationFunctionType.Exp,
                            scale=float(SCALE),
                        )

                    return f

                def u_ctx(u, t):
                    def f():
                        if "ctx" not in st:
                            st["ctx"] = [
                                apool.tile(
                                    [128, D], BF, tag=f"ctx{i}", name=f"ctx{i}", bufs=1
                                )
                                for i in range(TT)
                            ]
                        esU = es_tiles.pop((2 * u, t))
                        esL = es_tiles.pop((2 * u + 1, t))
                        ps_c = ppool.tile([128, 130], F32, tag="cx", bufs=2, name="ps_c")
                        nc.tensor.matmul(
                            ps_c[0:64, :],
                            lhsT=esU[:],
                            rhs=st["vb"][2 * u][:, t * 130 : (t + 1) * 130],
                            start=True,
                            stop=True,
                        )
                        nc.tensor.matmul(
                            ps_c[64:128, :],
                            lhsT=esL[:],
                            rhs=st["vb"][2 * u + 1][:, t * 130 : (t + 1) * 130],
                            start=True,
                            stop=True,
                        )
                        ps_b = ps_c[:].rearrange("p (b c) -> p b c", c=65)
                        rcp = apool.tile([128, 2], F32, tag="recip", name="rcp")
                        rc3 = rcp[:].rearrange("p (b c) -> p b c", c=1)
                        nc.vector.reciprocal(rc3, ps_b[:, :, 64:65])
                        ctx3 = st["ctx"][u].rearrange("p (h c) -> p h c", c=DH)
                        M = mybir.AluOpType.mult
                        # upper half: batch 2u, head order (h_even, h_odd)
                        nc.vector.tensor_tensor(
                            out=ctx3[0:64, 2 * t : 2 * t + 2, :],
                            in0=ps_b[0:64, :, 0:64],
                            in1=rc3[0:64].broadcast_to((64, 2, 64)),
                            op=M,
                        )
                        # lower half: batch 2u+1, head order (h_odd, h_even)
                        nc.vector.tensor_tensor(
                            out=ctx3[64:128, 2 * t : 2 * t + 2, :][:, ::-1, :],
                            in0=ps_b[64:128, :, 0:64],
                            in1=rc3[64:128].broadcast_to((64, 2, 64)),
                            op=M,
                        )

                    return f

                def u_ctxT(u):
                    def f():
                        if "cT" not in st:
                            st["cT"] = fpool.tile(
                                [128, KT * CHUNK], BF, tag="cT", name="cT", bufs=2
                            )
                        cT_blocks = st["cT"].rearrange("p (k c) -> p k c", c=CHUNK)
                        for g in range(2):
                            batched_transpose(
                                [
                                    st["ctx"][u][:, k * 128 : (k + 1) * 128]
                                    for k in range(g * 4, g * 4 + 4)
                                ],
                                cT_blocks[:, g * 4 : g * 4 + 4, u * 128 : (u + 1) * 128],
                            )

                    return f

                units = []
                ou = out_units(ch) if with_out else None
                for u in range(TT):
                    units.append(u_scores(2 * u, 0))
                    units.append(u_scores(2 * u + 1, 0))
                    for t in range(KT - 1):
                        units.append(u_scores(2 * u, t + 1))
                        units.append(u_scores(2 * u + 1, t + 1))
                        units.append(u_ctx(u, t))
                    units.append(u_ctx(u, KT - 1))
                    units.append(u_ctxT(u))
                    if ou is not None:
                        # this token-tile's output projection can start as
                        # soon as its ctxT landed — keeps the tail dense
                        units.extend(ou[u * 2 : u * 2 + 2])
                return units

            def out_units(ch):
                """Output projection + gelu + store for chunk ch (needs cT)."""
                tok0 = ch * CHUNK
                st = live[ch]

                def u_out(t, n):
                    def f():
                        cT = st["cT"]
                        ps = ppool.tile([128, CHUNK], F32, tag="proj", bufs=2, name="ps_o")
                        for k in range(KT):
                            nc.tensor.matmul(
                                ps[:],
                                lhsT=cT[:, k * CHUNK + t * 128 : k * CHUNK + (t + 1) * 128],
                                rhs=wt["wo"][k][:, n * 512 : (n + 1) * 512],
                                start=(k == 0),
                                stop=(k == KT - 1),
                            )
                        tmp = opool.tile([128, 512], F32, tag="obuf", name="tmp")
                        nc.vector.tensor_tensor(
                            out=tmp[:],
                            in0=ps[:],
                            in1=biases["wo"][:, n * 512 : (n + 1) * 512],
                            op=mybir.AluOpType.add,
                        )
                        og = opool.tile([128, 512], F32, tag="ogelu", name="og")
                        nc.scalar.activation(
                            out=og[:], in_=tmp[:], func=mybir.ActivationFunctionType.Gelu
                        )
                        nc.sync.dma_start(
                            out=out_d[
                                tok0 + t * 128 : tok0 + (t + 1) * 128,
                                n * 512 : (n + 1) * 512,
                            ],
                            in_=og[:],
                        )

                    return f

                return [u_out(t, n) for t in range(TT) for n in range(2)]

            # ---- emission ----
            identity = cpool.tile([128, 128], BF, tag="ident", name="identity")
            make_identity(nc, identity[:])
            consts["identity"] = identity
            stages = [stage_a_units(ch) for ch in range(NCH)]
            # prologue: first halves of wq + x(0) land first so the Q
            # projection's k=0..3 matmuls can start while the second halves
            # are still in flight; then chunk-0 projections interleaved with
            # the remaining weight loads and x(1)
            unit_load_weight("wq", 0, 2)()
            stages[0]["x_split"][0]()
            unit_load_weight("wq", 1, 2)()
            stages[0]["x_split"][1]()
            unit_biases()()
            unit_load_weight("wk")()
            for u in _interleave(stages[0]["q"], stages[1]["x"]):
                u()
            unit_load_weight("wv")()
            for u in stages[0]["k"]:
                u()
            unit_load_weight("wo")()
            for u in stages[0]["v"]:
                u()
            # steady state: window ch emits proj(ch) + x(ch+1) + attention
            # core of ch-1 + output projection of ch-2 (the delay keeps the
            # tail full of dense work to interleave with the final attention)
            for ch in range(1, NCH):
                dense = stages[ch]["proj"]
                if ch + 1 < NCH:
                    dense = _interleave(dense, stages[ch + 1]["x"])
                if ch - 2 >= 0:
                    dense = _interleave(dense, out_units(ch - 2))
                for u in _interleave(dense, attn_core_units(ch - 1)):
                    u()
                if ch - 2 >= 0:
                    live.pop(ch - 2)
            # tail: last chunk's attention (with its own output projection
            # inlined per token-tile) over the pending chunk's output
            # projection.
            pend = out_units(NCH - 2)
            for u in _interleave(attn_core_units(NCH - 1, with_out=True), pend):
                u()
            live.pop(NCH - 2)
            live.pop(NCH - 1)

    if split_waits:
        _split_multiwait(nc)
    return nc


_NC = None


def _get_nc():
    global _NC
    if _NC is None:
        _NC = build()
    return _NC


def _make_in_maps(inputs):
    x = np.asarray(inputs["x"], dtype=np.float32).astype(ml_dtypes.bfloat16)
    full = {}
    for nm in ("wq", "wk", "wv", "wo"):
        full[f"{nm}_w"] = np.ascontiguousarray(
            np.asarray(inputs[f"{nm}_w"], dtype=np.float32).astype(ml_dtypes.bfloat16)
        )
        full[f"{nm}_b"] = np.ascontiguousarray(
            np.asarray(inputs[f"{nm}_b"], dtype=np.float32)
        )
    in_maps = []
    for c in range(NCORES):
        # pre-transpose to feature-major [D, NTOK] so no on-chip x transpose
        xc = np.ascontiguousarray(x[c * BL : (c + 1) * BL].reshape(NTOK, D).T)
        m = {"x": xc}
        m.update(full)
        in_maps.append(m)
    return in_maps


def kernel(**inputs):
    nc = _get_nc()
    res = run_bass_kernel_spmd(
        nc, _make_in_maps(inputs), core_ids=list(range(NCORES))
    ).results
    parts = [res[c]["out"].reshape(BL, 8, 8, D) for c in range(NCORES)]
    return np.concatenate(parts, axis=0)


def kernel_profiled(**inputs):
    """Like kernel() but requests an NTFF trace; returns (out, exec_time_ns, raw)."""
    nc = _get_nc()
    r = run_bass_kernel_spmd(
        nc, _make_in_maps(inputs), core_ids=list(range(NCORES)), trace=True
    )
    parts = [r.results[c]["out"].reshape(BL, 8, 8, D) for c in range(NCORES)]
    return np.concatenate(parts, axis=0), r.exec_time_ns, r
